# revision 1
# baseline (speedup 1.0000x reference)
"""Multi-head attention (softmax over the HEADS axis) on 8 trn2 NeuronCores.

Reference math (B=2, S=2048, D=512, H=8, Dk=64):
    q = split_heads(Q @ w_q.T + b_q)          # [B,H,S,Dk]
    scores = q @ k.T / sqrt(Dk)               # [B,H,Sq,Sk]
    probs = softmax(scores, axis=1)           # softmax over H (source quirk!)
    attn = probs @ v                          # [B,H,Sq,Dk]
    out = concat_heads(attn) @ w_o.T + b_o    # [B,S,D]

Because softmax is over H, it is local to each (b, sq, sk) position: sharding
over (batch x query rows) needs no cross-core communication.  Core c handles
batch c//4, query rows (c%4)*512 .. +512, with all 8 heads resident.

All matmul operands are bf16 (fp32 matmul runs LOW_HIGH = 2x instructions);
PSUM accumulation stays fp32.  Steady state is elementwise-bound: per kj tile
the budget is ~4 us on each of ACT (exp), DVE and GpSimd (head-sum tree,
reciprocal, normalize), so that work is split across all three.

Layouts (contraction dim always on SBUF partitions):
  qt/kt/vt  [128, 4, S*]  = X.T      (din = chunk*128 + p), bf16
  w*t       [128, 4, 512] = W.T      (din = chunk*128 + p), bf16
  qTs/kTs   [128, 4, S*]  = proj.T   (dout = m*128 + p), bf16
  vs        [128, 16, 512] = v natural (kj on partitions), bf16
  scores    psum [kj=128, 2, 512qi] per head pair -> exp -> softmax over h
  attn      psum [2*64=128 d, 512 qi] per head-pair, accumulated over kj tiles
  out       [qi, 512] natural, fp32
"""

import numpy as np

B, S, D, H, DK = 2, 2048, 512, 8, 64
NCORES = 8
CPB = NCORES // B          # cores per batch
QI = S // CPB              # query rows per core (512)
KJT = 128                  # kj tile (partition dim of scores)
NKJ = S // KJT             # 16 kj tiles
NC_, CH = 128, 4           # partitions, din chunks
SCALE = 1.0 / np.sqrt(DK)  # folded into exp activation


def _chunk(x, dt):
    """[512, F] -> [128, 4, F] with row = chunk*128 + p."""
    f = x.shape[1]
    return np.ascontiguousarray(
        np.ascontiguousarray(x).reshape(CH, NC_, f).transpose(1, 0, 2)
    ).astype(dt)


def _build(with_bias):
    from contextlib import ExitStack

    import concourse.bass as bass
    import concourse.mybir as mybir
    import concourse.tile as tile
    from concourse import bacc

    fp32 = mybir.dt.float32
    bf16 = mybir.dt.bfloat16

    nc = bacc.Bacc(
        "TRN2",
        target_bir_lowering=False,
        debug=False,
        enable_asserts=False,
        num_devices=NCORES,
    )

    def din(name, shape):
        return nc.dram_tensor(name, shape, bf16, kind="ExternalInput").ap()

    qt_d = din("qt", [NC_, CH, QI])
    kt_d = din("kt", [NC_, CH, S])
    vt_d = din("vt", [NC_, CH, S])
    w_d = {n: din(n, [NC_, CH, D]) for n in ("wqt", "wkt", "wvt", "wot")}
    if with_bias:
        b_d = {n: din(n, [1, D]) for n in ("bq", "bk", "bv", "bo")}
    out_d = nc.dram_tensor("out", [QI, D], fp32, kind="ExternalOutput").ap()

    with tile.TileContext(nc) as tc, ExitStack() as ctx:
        wpool = ctx.enter_context(tc.tile_pool(name="wts", bufs=2))
        raw = ctx.enter_context(tc.tile_pool(name="raw", bufs=5))
        acts = ctx.enter_context(tc.tile_pool(name="acts", bufs=1))
        sm = ctx.enter_context(tc.tile_pool(name="sm", bufs=4))
        pp = ctx.enter_context(tc.tile_pool(name="pp", bufs=8))
        ps = ctx.enter_context(tc.tile_pool(name="ps", bufs=2, space="PSUM"))
        psa = ctx.enter_context(tc.tile_pool(name="psa", bufs=4, space="PSUM"))

        qTs = acts.tile([NC_, CH, QI], bf16, tag="qTs")
        kTs = acts.tile([NC_, CH, S], bf16, tag="kTs")
        vs = acts.tile([NC_, NKJ, D], bf16, tag="vs")
        attnT = acts.tile([NC_, CH, QI], bf16, tag="attnT")
        outsb = acts.tile([NC_, CH, D], fp32, tag="outsb")

        if with_bias:
            ones = acts.tile([1, D], bf16, tag="ones")
            nc.vector.memset(ones, 1.0)
            brow = {}
            for n in ("bq", "bk", "bv", "bo"):
                brow[n] = acts.tile([1, D], bf16, tag=n, name=n)
                nc.sync.dma_start(out=brow[n], in_=b_d[n])

        wsb = {}
        weng = [nc.sync, nc.sync, nc.sync, nc.sync]
        for ei, n in enumerate(("wqt", "wkt", "wvt", "wot")):
            wsb[n] = wpool.tile([NC_, CH, D], bf16, tag="w", name=n)
            weng[ei].dma_start(out=wsb[n], in_=w_d[n])

        def bias_mm(pt_ap, bname, col_slice):
            """rank-1 bias init: psum = bias-row (x) ones-row (or flipped)."""
            if col_slice is not None:  # bias along partitions
                lhsT = brow[bname][:, col_slice]
                rhs = ones[:, : pt_ap.shape[-1]]
            else:  # bias along free dim
                lhsT = ones[:, :128]
                rhs = brow[bname]
            nc.tensor.matmul(pt_ap, lhsT=lhsT, rhs=rhs, start=True, stop=False)

        # ---------------- projections ----------------
        # Q: qT[dout, qi] = wqT[din,dout].T @ QT[din, qi]
        qraw = raw.tile([NC_, CH, QI], bf16, tag="raw")
        nc.sync.dma_start(out=qraw, in_=qt_d)
        for m in range(4):
            pt = psa.tile([NC_, 1, 512], fp32, tag="attn")
            if with_bias:
                bias_mm(pt[:, 0, :QI], "bq", slice(m * 128, (m + 1) * 128))
            for c in range(CH):
                nc.tensor.matmul(
                    pt[:, 0, :QI],
                    lhsT=wsb["wqt"][:, c, m * 128 : (m + 1) * 128],
                    rhs=qraw[:, c, :],
                    start=(c == 0 and not with_bias),
                    stop=(c == CH - 1),
                )
            if m % 2 == 0:
                nc.scalar.copy(qTs[:, m, :], pt[:, 0, :QI])
            else:
                nc.vector.tensor_copy(qTs[:, m, :], pt[:, 0, :QI])

        # K: kT[dout, kj]
        kraw = [raw.tile([NC_, S], bf16, tag="raw", name=f"kraw{c}") for c in range(CH)]
        for c in range(CH):
            weng[c].dma_start(out=kraw[c], in_=kt_d[:, c, :])
        for m in range(4):
            for kc in range(4):
                pt = psa.tile([NC_, 1, 512], fp32, tag="attn")
                if with_bias:
                    bias_mm(pt[:, 0, :], "bk", slice(m * 128, (m + 1) * 128))
                for c in range(CH):
                    nc.tensor.matmul(
                        pt[:, 0, :],
                        lhsT=wsb["wkt"][:, c, m * 128 : (m + 1) * 128],
                        rhs=kraw[c][:, kc * 512 : (kc + 1) * 512],
                        start=(c == 0 and not with_bias),
                        stop=(c == CH - 1),
                    )
                if kc % 2 == 0:
                    nc.scalar.copy(
                        kTs[:, m, kc * 512 : (kc + 1) * 512], pt[:, 0, :]
                    )
                else:
                    nc.vector.tensor_copy(
                        kTs[:, m, kc * 512 : (kc + 1) * 512], pt[:, 0, :]
                    )

        # V: v natural [kj, dout] = VT[din,kj].T @ wvT[din,dout]
        vraw = [raw.tile([NC_, S], bf16, tag="raw", name=f"vraw{c}") for c in range(CH)]
        for c in range(CH):
            weng[c].dma_start(out=vraw[c], in_=vt_d[:, c, :])
        for t in range(NKJ):
            pt = psa.tile([NC_, 1, 512], fp32, tag="attn")
            if with_bias:
                bias_mm(pt[:, 0, :], "bv", None)
            for c in range(CH):
                nc.tensor.matmul(
                    pt[:, 0, :],
                    lhsT=vraw[c][:, t * 128 : (t + 1) * 128],
                    rhs=wsb["wvt"][:, c, :],
                    start=(c == 0 and not with_bias),
                    stop=(c == CH - 1),
                )
            nc.scalar.copy(vs[:, t, :], pt[:, 0, :])

        # ---------------- attention ----------------
        # attn psum: tile dc holds heads 2dc (p 0..63), 2dc+1 (p 64..127)
        at = [psa.tile([NC_, 512], fp32, tag="attn", name=f"at{i}") for i in range(4)]

        def emit_attn(td, prs):
            for h in range(H):
                po = (h % 2) * 64
                nc.tensor.matmul(
                    at[h // 2][po : po + 64, :QI],
                    lhsT=vs[:, td, h * 64 : (h + 1) * 64],
                    rhs=prs[h // 4][:, h % 4, :],
                    start=(td == 0),
                    stop=(td == NKJ - 1),
                )

        LAG = 3
        pending = []
        for t in range(NKJ):
            exp_t = sm.tile([NC_, H, QI], bf16, tag="exp", bufs=6)
            for m in range(4):
                spt = ps.tile([NC_, 2, 512], fp32, tag="ps")
                for j in range(2):
                    po = j * 64
                    nc.tensor.matmul(
                        spt[:, j, :QI],
                        lhsT=kTs[po : po + 64, m, t * 128 : (t + 1) * 128],
                        rhs=qTs[po : po + 64, m, :],
                        start=True,
                        stop=True,
                    )
                nc.scalar.activation(
                    exp_t[:, 2 * m : 2 * m + 2, :],
                    spt[:, :, :],
                    mybir.ActivationFunctionType.Exp,
                    scale=SCALE,
                )

            # head-sum tree, split across gpsimd (slow) and DVE to balance:
            # gpsimd: L1a full + L1b first half; DVE: L1b second half, L2, L3
            s01 = sm.tile([NC_, 2, QI], bf16, tag="s01")
            s23 = sm.tile([NC_, 2, QI], bf16, tag="s23")
            nc.gpsimd.tensor_add(s01, exp_t[:, 0:2, :], exp_t[:, 2:4, :])
            nc.gpsimd.tensor_add(s23, exp_t[:, 4:6, :], exp_t[:, 6:8, :])
            nc.vector.tensor_add(s01, s01, s23)
            ssum = sm.tile([NC_, QI], fp32, tag="ssum")
            nc.vector.tensor_add(ssum, s01[:, 0, :], s01[:, 1, :])
            # fast reciprocal writing bf16 directly (DVE converts on the
            # final write; the fp32 bit-trick only needs the fp32 *input*)
            from concourse.dve_ops import (
                RECIP_APPROX_FAST_CONSTS as _RC,
                RECIPROCAL_APPROX_FAST as _RF,
            )
            r = sm.tile([NC_, QI], bf16, tag="r")
            nc.vector._custom_dve(
                _RF, out=r, in0=ssum, s0=_RC["s0"], s1=_RC["s1"], imm2=_RC["imm2"]
            )

            # normalize per head: plain contiguous operands keep DVE 2x mode;
            # one head's mul goes to gpsimd to shave the DVE stream
            prs = []
            for g in range(2):
                pr = pp.tile([NC_, 4, QI], bf16, tag="probs")
                for jj in range(4):
                    nc.vector.tensor_mul(
                        pr[:, jj, :], exp_t[:, 4 * g + jj, :], r
                    )
                prs.append(pr)

            # attn matmuls run LAG tiles behind (probs already ready -> PE
            # never stalls mid-stream on the softmax chain)
            pending.append((t, prs))
            if len(pending) > LAG:
                emit_attn(*pending.pop(0))

        for td, prs in pending:
            emit_attn(td, prs)

        for dc in range(4):
            eng = nc.vector if dc % 2 == 0 else nc.scalar
            if dc % 2 == 0:
                nc.vector.tensor_copy(attnT[:, dc, :], at[dc][:, :QI])
            else:
                nc.scalar.copy(attnT[:, dc, :], at[dc][:, :QI])

        # ---------------- output projection ----------------
        for m in range(4):
            ot = psa.tile([NC_, 512], fp32, tag="attn")
            if with_bias:
                bias_mm(ot, "bo", None)
            for c in range(CH):
                nc.tensor.matmul(
                    ot,
                    lhsT=attnT[:, c, m * 128 : (m + 1) * 128],
                    rhs=wsb["wot"][:, c, :],
                    start=(c == 0 and not with_bias),
                    stop=(c == CH - 1),
                )
            if m % 2 == 0:
                nc.scalar.copy(outsb[:, m, :], ot)
            else:
                nc.vector.tensor_copy(outsb[:, m, :], ot)
            nc.sync.dma_start(
                out=out_d.rearrange("(m p) o -> p m o", p=NC_)[:, m, :],
                in_=outsb[:, m, :],
            )

    nc.compile()
    return nc


_CACHE = {}


def kernel(Q, K, V, w_q, b_q, w_k, b_k, w_v, b_v, w_o, b_o, _trace=False):
    import ml_dtypes
    from concourse import bass_utils

    bf = ml_dtypes.bfloat16
    Q = np.asarray(Q, np.float32)
    K = np.asarray(K, np.float32)
    V = np.asarray(V, np.float32)
    with_bias = any(
        np.any(np.asarray(b) != 0) for b in (b_q, b_k, b_v, b_o)
    )

    if ("nc", with_bias) not in _CACHE:
        _CACHE[("nc", with_bias)] = _build(with_bias)
    nc = _CACHE[("nc", with_bias)]

    wmaps = {
        "wqt": _chunk(np.asarray(w_q, np.float32).T, bf),
        "wkt": _chunk(np.asarray(w_k, np.float32).T, bf),
        "wvt": _chunk(np.asarray(w_v, np.float32).T, bf),
        "wot": _chunk(np.asarray(w_o, np.float32).T, bf),
    }
    if with_bias:
        for n, b in (("bq", b_q), ("bk", b_k), ("bv", b_v), ("bo", b_o)):
            wmaps[n] = np.ascontiguousarray(
                np.asarray(b, np.float32).reshape(1, D)
            ).astype(bf)

    in_maps = []
    for c in range(NCORES):
        b = c // CPB
        s0 = (c % CPB) * QI
        in_maps.append(
            dict(
                wmaps,
                qt=_chunk(Q[b, s0 : s0 + QI, :].T, bf),
                kt=_chunk(K[b].T, bf),
                vt=_chunk(V[b].T, bf),
            )
        )

    res = bass_utils.run_bass_kernel_spmd(
        nc, in_maps, core_ids=list(range(NCORES)), trace=_trace
    )

    out = np.empty((B, S, D), np.float32)
    for c in range(NCORES):
        b = c // CPB
        s0 = (c % CPB) * QI
        out[b, s0 : s0 + QI, :] = res.results[c]["out"]
    if _trace:
        kernel._last_results = res
    return out



# revision 14
# speedup vs baseline: 1.0995x; 1.0995x over previous
"""Multi-head attention (softmax over the HEADS axis) on 8 trn2 NeuronCores.

Reference math (B=2, S=2048, D=512, H=8, Dk=64):
    q = split_heads(Q @ w_q.T + b_q)          # [B,H,S,Dk]
    scores = q @ k.T / sqrt(Dk)               # [B,H,Sq,Sk]
    probs = softmax(scores, axis=1)           # softmax over H (source quirk!)
    attn = probs @ v                          # [B,H,Sq,Dk]
    out = concat_heads(attn) @ w_o.T + b_o    # [B,S,D]

Because softmax is over H, it is local to each (b, sq, sk) position: sharding
over (batch x query rows) needs no cross-core communication.  Core c handles
batch c//4, query rows (c%4)*512 .. +512, with all 8 heads resident.

Pipelined single-loop design: K/V projections for later kj tiles are emitted
INSIDE the attention loop (V tile t+4 and one K dout-block of the next column
block per iteration), so the PE never goes idle (stays at warm 2.4 GHz clock)
and the projection phase overlaps the softmax elementwise chain.  Per kj tile:
  PE : 8 score MMs + 8 PV MMs (lagged) + 4 V-proj + 4 K-proj MMs
  ACT: 4 exp activations [128,2,512] (PSUM->SBUF, (N+352)/1.2 ns each)
  DVE: s4 = e[0:4]+e[4:8]; ssum = s2[0]+s2[1] (bf16); fast-recip; 2 broadcast
       normalize muls [128,4,512] (r stride-0 over the head dim)
  GpS: s2 = s4[0:2]+s4[2:4]; V-copy + K-copy (PSUM->SBUF casts)
PSUM: 4 banks attn accumulators + 4 banks (bufs=2 x [128,2,512]) shared by
score m-pairs and the packed (V-tile, K-block) projection outputs.

Layouts (contraction dim always on SBUF partitions):
  qt/kt/vt  [128, 4, S*]  = X.T      (din = chunk*128 + p), bf16
  w*t       [128, 4, 512] = W.T      (din = chunk*128 + p), bf16
  qTs/kTs   [128, 4, S*]  = proj.T   (dout = m*128 + p), bf16
  vs        [128, 16, 512] = v natural (kj on partitions), bf16
  scores    psum [kj=128, 2, 512qi] per head pair -> exp -> softmax over h
  attn      psum [2*64=128 d, 512 qi] per head-pair, accumulated over kj tiles
  out       [qi, 512] natural, fp32
"""

import numpy as np

B, S, D, H, DK = 2, 2048, 512, 8, 64
NCORES = 8
CPB = NCORES // B          # cores per batch
QI = S // CPB              # query rows per core (512)
KJT = 128                  # kj tile (partition dim of scores)
NKJ = S // KJT             # 16 kj tiles
NC_, CH = 128, 4           # partitions, din chunks
SCALE = 1.0 / np.sqrt(DK)  # folded into exp activation
LAG = 3                    # PV matmuls run LAG kj tiles behind the softmax


def _chunk(x, dt):
    """[512, F] -> [128, 4, F] with row = chunk*128 + p."""
    f = x.shape[1]
    return np.ascontiguousarray(
        np.ascontiguousarray(x).reshape(CH, NC_, f).transpose(1, 0, 2)
    ).astype(dt)


def _build(with_bias):
    from contextlib import ExitStack

    import concourse.bass as bass
    import concourse.mybir as mybir
    import concourse.tile as tile
    from concourse import bacc
    from concourse.dve_ops import (
        RECIP_APPROX_FAST_CONSTS as _RC,
        RECIPROCAL_APPROX_FAST as _RF,
    )

    fp32 = mybir.dt.float32
    bf16 = mybir.dt.bfloat16

    nc = bacc.Bacc(
        "TRN2",
        target_bir_lowering=False,
        debug=False,
        enable_asserts=False,
        num_devices=NCORES,
    )

    def din(name, shape):
        return nc.dram_tensor(name, shape, bf16, kind="ExternalInput").ap()

    qt_d = din("qt", [NC_, CH, QI])
    kt_d = din("kt", [NC_, CH, S])
    vt_d = din("vt", [NC_, CH, S])
    w_d = {n: din(n, [NC_, CH, D]) for n in ("wqt", "wkt", "wvt", "wot")}
    if with_bias:
        b_d = {n: din(n, [1, D]) for n in ("bq", "bk", "bv", "bo")}
    out_d = nc.dram_tensor("out", [QI, D], fp32, kind="ExternalOutput").ap()

    with tile.TileContext(nc) as tc, ExitStack() as ctx:
        acts = ctx.enter_context(tc.tile_pool(name="acts", bufs=1))
        sm = ctx.enter_context(tc.tile_pool(name="sm", bufs=2))
        pp = ctx.enter_context(tc.tile_pool(name="pp", bufs=2 * (LAG + 1)))
        ps = ctx.enter_context(tc.tile_pool(name="ps", bufs=2, space="PSUM"))
        psa = ctx.enter_context(tc.tile_pool(name="psa", bufs=1, space="PSUM"))

        # ---- persistent SBUF tiles ----
        # kTs per 512-col block and vs per kj tile are separate tiles so the
        # in-loop projection writes can never serialize unrelated readers
        qTs = acts.tile([NC_, CH, QI], bf16, tag="qTs")
        kTsb = [
            acts.tile([NC_, CH, 512], bf16, tag=f"kTs{b_}", name=f"kTs{b_}")
            for b_ in range(4)
        ]
        vst = [
            acts.tile([NC_, D], bf16, tag=f"vs{i}", name=f"vs{i}")
            for i in range(NKJ)
        ]
        attnT = acts.tile([NC_, CH, QI], bf16, tag="attnT")
        outsb = acts.tile([NC_, CH, D], fp32, tag="outsb")
        qraw = acts.tile([NC_, CH, QI], bf16, tag="qraw")
        kraw = [
            acts.tile([NC_, S], bf16, tag=f"kraw{c}", name=f"kraw{c}")
            for c in range(CH)
        ]
        vraw = [
            acts.tile([NC_, S], bf16, tag=f"vraw{c}", name=f"vraw{c}")
            for c in range(CH)
        ]
        wsb = {
            n: acts.tile([NC_, CH, D], bf16, tag=n, name=n)
            for n in ("wqt", "wkt", "wvt", "wot")
        }

        if with_bias:
            ones = acts.tile([1, D], bf16, tag="ones")
            nc.vector.memset(ones, 1.0)
            brow = {}
            for n in ("bq", "bk", "bv", "bo"):
                brow[n] = acts.tile([1, D], bf16, tag=n, name=n)
                nc.sync.dma_start(out=brow[n], in_=b_d[n])

        # ---- prologue DMAs, spread across engine queues for parallelism ----
        nc.sync.dma_start(out=wsb["wqt"], in_=w_d["wqt"])
        nc.sync.dma_start(out=qraw, in_=qt_d)
        nc.sync.dma_start(out=wsb["wkt"], in_=w_d["wkt"])
        nc.scalar.dma_start(out=kraw[0], in_=kt_d[:, 0, :])
        nc.scalar.dma_start(out=kraw[1], in_=kt_d[:, 1, :])
        nc.gpsimd.dma_start(out=kraw[2], in_=kt_d[:, 2, :])
        nc.gpsimd.dma_start(out=kraw[3], in_=kt_d[:, 3, :])
        nc.scalar.dma_start(out=wsb["wvt"], in_=w_d["wvt"])
        nc.scalar.dma_start(out=vraw[0], in_=vt_d[:, 0, :])
        nc.gpsimd.dma_start(out=vraw[1], in_=vt_d[:, 1, :])
        nc.gpsimd.dma_start(out=vraw[2], in_=vt_d[:, 2, :])
        nc.gpsimd.dma_start(out=vraw[3], in_=vt_d[:, 3, :])
        nc.sync.dma_start(out=wsb["wot"], in_=w_d["wot"])

        def bias_mm(pt_ap, bname, col_slice):
            """rank-1 bias init: psum = bias-row (x) ones-row (or flipped)."""
            if col_slice is not None:  # bias along partitions
                lhsT = brow[bname][:, col_slice]
                rhs = ones[:, : pt_ap.shape[-1]]
            else:  # bias along free dim
                lhsT = ones[:, :128]
                rhs = brow[bname]
            nc.tensor.matmul(pt_ap, lhsT=lhsT, rhs=rhs, start=True, stop=False)

        # ---------------- prologue projections ----------------
        # Q: qT[dout, qi] = wqT[din,dout].T @ QT[din, qi]; two m per psum tile
        for mp in range(2):
            pt = ps.tile([NC_, 2, 512], fp32, tag="sc", name=f"qp{mp}")
            for j in range(2):
                m = 2 * mp + j
                if with_bias:
                    bias_mm(pt[:, j, :QI], "bq", slice(m * 128, (m + 1) * 128))
                for c in range(CH):
                    nc.tensor.matmul(
                        pt[:, j, :QI],
                        lhsT=wsb["wqt"][:, c, m * 128 : (m + 1) * 128],
                        rhs=qraw[:, c, :],
                        start=(c == 0 and not with_bias),
                        stop=(c == CH - 1),
                    )
            if mp == 0:
                nc.scalar.copy(qTs[:, 0:2, :], pt[:, :, :QI])
            else:
                nc.vector.tensor_copy(qTs[:, 2:4, :], pt[:, :, :QI])

        # K block 0 (kj cols 0..511): kT[dout, kj]
        for mp in range(2):
            pt = ps.tile([NC_, 2, 512], fp32, tag="sc", name=f"kp{mp}")
            for j in range(2):
                m = 2 * mp + j
                if with_bias:
                    bias_mm(pt[:, j, :], "bk", slice(m * 128, (m + 1) * 128))
                for c in range(CH):
                    nc.tensor.matmul(
                        pt[:, j, :],
                        lhsT=wsb["wkt"][:, c, m * 128 : (m + 1) * 128],
                        rhs=kraw[c][:, 0:512],
                        start=(c == 0 and not with_bias),
                        stop=(c == CH - 1),
                    )
            if mp == 0:
                nc.scalar.copy(kTsb[0][:, 0:2, :], pt)
            else:
                nc.vector.tensor_copy(kTsb[0][:, 2:4, :], pt)

        # V tiles 0..3: v natural [kj, dout] = VT[din,kj].T @ wvT[din,dout]
        for g in range(2):
            pt = ps.tile([NC_, 2, 512], fp32, tag="sc", name=f"vp{g}")
            for j in range(2):
                t = 2 * g + j
                if with_bias:
                    bias_mm(pt[:, j, :], "bv", None)
                for c in range(CH):
                    nc.tensor.matmul(
                        pt[:, j, :],
                        lhsT=vraw[c][:, t * 128 : (t + 1) * 128],
                        rhs=wsb["wvt"][:, c, :],
                        start=(c == 0 and not with_bias),
                        stop=(c == CH - 1),
                    )
            for j in range(2):
                t = 2 * g + j
                if g == 0:
                    nc.vector.tensor_copy(vst[t], pt[:, j, :])
                else:
                    nc.scalar.copy(vst[t], pt[:, j, :])

        # ---------------- fused attention + pipelined K/V projection ----------
        at = [
            psa.tile([NC_, 512], fp32, tag=f"at{i}", name=f"at{i}")
            for i in range(4)
        ]

        def emit_pv(td, prs, m):
            """PV matmuls for heads 2m, 2m+1 of kj tile td."""
            for h in (2 * m, 2 * m + 1):
                po = (h % 2) * 64
                nc.tensor.matmul(
                    at[h // 2][po : po + 64, :QI],
                    lhsT=vst[td][:, h * 64 : (h + 1) * 64],
                    rhs=prs[h // 4][:, h % 4, :],
                    start=(td == 0),
                    stop=(td == NKJ - 1),
                )

        pending = []
        for t in range(NKJ):
            exp_t = sm.tile([NC_, H, QI], bf16, tag="exp", bufs=3)
            pv = pending.pop(0) if len(pending) >= LAG else None

            # pipelined projections FIRST in this iteration's PE stream:
            # V tile t+4 and K dout-block (t%4) of column block t//4+1,
            # packed into one [128,2,512] psum tile; copies go to gpsimd
            # early so the psum slot frees well before it is needed again
            if t < NKJ - CH:
                vt_i = t + CH
                km, kb = t % 4, t // 4 + 1
                kvt = ps.tile([NC_, 2, 512], fp32, tag="sc", name=f"kv{t}")
                if with_bias:
                    bias_mm(kvt[:, 0, :], "bv", None)
                for c in range(CH):
                    nc.tensor.matmul(
                        kvt[:, 0, :],
                        lhsT=vraw[c][:, vt_i * 128 : (vt_i + 1) * 128],
                        rhs=wsb["wvt"][:, c, :],
                        start=(c == 0 and not with_bias),
                        stop=(c == CH - 1),
                    )
                if with_bias:
                    bias_mm(kvt[:, 1, :], "bk", slice(km * 128, (km + 1) * 128))
                for c in range(CH):
                    nc.tensor.matmul(
                        kvt[:, 1, :],
                        lhsT=wsb["wkt"][:, c, km * 128 : (km + 1) * 128],
                        rhs=kraw[c][:, kb * 512 : (kb + 1) * 512],
                        start=(c == 0 and not with_bias),
                        stop=(c == CH - 1),
                    )
                nc.vector.tensor_copy(vst[vt_i], kvt[:, 0, :])
                nc.scalar.copy(kTsb[kb][:, km, :], kvt[:, 1, :])

            for m in range(4):
                spt = ps.tile([NC_, 2, 512], fp32, tag="sc", name=f"s{t}_{m}")
                for j in range(2):
                    po = j * 64
                    nc.tensor.matmul(
                        spt[:, j, :QI],
                        lhsT=kTsb[t // 4][
                            po : po + 64, m, (t % 4) * 128 : (t % 4 + 1) * 128
                        ],
                        rhs=qTs[po : po + 64, m, :],
                        start=True,
                        stop=True,
                    )
                if pv is not None:
                    emit_pv(pv[0], pv[1], m)
                nc.scalar.activation(
                    exp_t[:, 2 * m : 2 * m + 2, :],
                    spt[:, :, :],
                    mybir.ActivationFunctionType.Exp,
                    scale=SCALE,
                )

            # head-sum tree: DVE L1 -> GpSimd L2 -> DVE L3 (bf16 throughout)
            s4 = sm.tile([NC_, 4, QI], bf16, tag="s4")
            nc.vector.tensor_add(s4, exp_t[:, 0:4, :], exp_t[:, 4:8, :])
            s2 = sm.tile([NC_, 2, QI], bf16, tag="s2")
            nc.gpsimd.tensor_add(s2, s4[:, 0:2, :], s4[:, 2:4, :])
            ssum = sm.tile([NC_, QI], bf16, tag="ssum")
            nc.vector.tensor_add(ssum, s2[:, 0, :], s2[:, 1, :])
            # fast reciprocal; bf16 in/out (DVE upconverts the read, and the
            # bit-trick seed only needs the fp32 *pipeline* representation)
            r = sm.tile([NC_, QI], bf16, tag="r")
            nc.vector._custom_dve(
                _RF, out=r, in0=ssum, s0=_RC["s0"], s1=_RC["s1"], imm2=_RC["imm2"]
            )

            # normalize: two [128,4,512] muls with r broadcast over heads
            prs = []
            for g in range(2):
                pr = pp.tile([NC_, 4, QI], bf16, tag="probs", name=f"pr{t}_{g}")
                nc.vector.tensor_mul(
                    pr,
                    exp_t[:, 4 * g : 4 * g + 4, :],
                    r[:, None, :].broadcast_to([NC_, 4, QI]),
                )
                prs.append(pr)
            pending.append((t, prs))

        # drain remaining PV tiles
        for td, prs in pending:
            for m in range(4):
                emit_pv(td, prs, m)

        for dc in range(4):
            if dc % 2 == 0:
                nc.vector.tensor_copy(attnT[:, dc, :], at[dc][:, :QI])
            else:
                nc.scalar.copy(attnT[:, dc, :], at[dc][:, :QI])

        # ---------------- output projection ----------------
        for mp in range(2):
            ot = ps.tile([NC_, 2, 512], fp32, tag="sc", name=f"op{mp}")
            for j in range(2):
                m = 2 * mp + j
                if with_bias:
                    bias_mm(ot[:, j, :], "bo", None)
                for c in range(CH):
                    nc.tensor.matmul(
                        ot[:, j, :],
                        lhsT=attnT[:, c, m * 128 : (m + 1) * 128],
                        rhs=wsb["wot"][:, c, :],
                        start=(c == 0 and not with_bias),
                        stop=(c == CH - 1),
                    )
            if mp == 0:
                nc.scalar.copy(outsb[:, 0:2, :], ot)
            else:
                nc.vector.tensor_copy(outsb[:, 2:4, :], ot)
            nc.sync.dma_start(
                out=out_d.rearrange("(m p) o -> p m o", p=NC_)[
                    :, 2 * mp : 2 * mp + 2, :
                ],
                in_=outsb[:, 2 * mp : 2 * mp + 2, :],
            )

    nc.compile()
    return nc


_CACHE = {}


def kernel(Q, K, V, w_q, b_q, w_k, b_k, w_v, b_v, w_o, b_o, _trace=False):
    import ml_dtypes
    from concourse import bass_utils

    bf = ml_dtypes.bfloat16
    Q = np.asarray(Q, np.float32)
    K = np.asarray(K, np.float32)
    V = np.asarray(V, np.float32)
    with_bias = any(
        np.any(np.asarray(b) != 0) for b in (b_q, b_k, b_v, b_o)
    )

    if ("nc", with_bias) not in _CACHE:
        _CACHE[("nc", with_bias)] = _build(with_bias)
    nc = _CACHE[("nc", with_bias)]

    wmaps = {
        "wqt": _chunk(np.asarray(w_q, np.float32).T, bf),
        "wkt": _chunk(np.asarray(w_k, np.float32).T, bf),
        "wvt": _chunk(np.asarray(w_v, np.float32).T, bf),
        "wot": _chunk(np.asarray(w_o, np.float32).T, bf),
    }
    if with_bias:
        for n, b in (("bq", b_q), ("bk", b_k), ("bv", b_v), ("bo", b_o)):
            wmaps[n] = np.ascontiguousarray(
                np.asarray(b, np.float32).reshape(1, D)
            ).astype(bf)

    in_maps = []
    for c in range(NCORES):
        b = c // CPB
        s0 = (c % CPB) * QI
        in_maps.append(
            dict(
                wmaps,
                qt=_chunk(Q[b, s0 : s0 + QI, :].T, bf),
                kt=_chunk(K[b].T, bf),
                vt=_chunk(V[b].T, bf),
            )
        )

    res = bass_utils.run_bass_kernel_spmd(
        nc, in_maps, core_ids=list(range(NCORES)), trace=_trace
    )

    out = np.empty((B, S, D), np.float32)
    for c in range(NCORES):
        b = c // CPB
        s0 = (c % CPB) * QI
        out[b, s0 : s0 + QI, :] = res.results[c]["out"]
    if _trace:
        kernel._last_results = res
    return out
